# revision 7
# baseline (speedup 1.0000x reference)
"""Trainium2 SPMD kernel for: y = BatchNorm1d(x @ sign(w).T + bias) * gamma + beta.

Sharding: data-parallel over the batch dim across 8 NeuronCores; the
(binarized) weight is replicated.  BatchNorm batch statistics use
on-device AllReduces of per-shard (sum_y, sum_y2).

Design (v3, output-stationary):
  - The matmul runs with the OUTPUT dim on PSUM partitions: lhsT = sign(w)
    [k, o] (stationary, fp8 +-1 exact), rhs = x^T [k, b] (moving, bf16).
    Host pre-transposes x and pre-binarizes w, so no on-device
    preprocessing and no casting DMAs.
  - x (8.4 MB bf16) is fully SBUF-resident after one load pass; weights
    are 2.1 MB fp8.  The PE never starves after startup.
  - With o on partitions, BN sums are free-dim reductions fused into the
    PSUM drain: DVE does copy+sum(y) (tensor_scalar + accum_out), the
    scalar engine does square+sum(y^2) - no tensor-engine stats matmuls.
  - Collectives serialize on the TOPSP stream (~12us each regardless of
    size) and can't start before the all-core start barrier, so stats go
    out in 4 grouped AllReduces triggered as blocks complete; the chain
    drains during compute and only the small last AR (ob 7) is exposed
    in the tail.  Coefficient math is placed so it never sits in an
    engine FIFO ahead of pending PSUM drains.
  - The linear bias cancels inside BatchNorm and is never applied.
  - Output is stored [o, b] bf16 and transposed/cast on the host.
"""

import os
import sys

sys.path.insert(0, "/opt/trn_rl_repo")

import numpy as np
import ml_dtypes

import concourse.bacc as bacc
import concourse.mybir as mybir
import concourse.tile as tile
from concourse import bass_utils

N_CORES = 8
B_TOT = 16384
D_IN = 2048
D_OUT = 1024
B_SH = B_TOT // N_CORES          # 2048 batch rows per core
KT = D_IN // 128                 # 16 contraction stripes
OB = D_OUT // 128                # 8 output blocks (PSUM partition dim)
BB = B_SH // 512                 # 4 batch blocks (PSUM free dim)
OG = 4                           # weight groups of 256 outputs
BN_EPS = 1e-5

# AllReduce groups: triggered after the last ob of each group completes.
GROUPS = [(0, 1, 2), (3, 4), (5, 6), (7,)]

F32 = mybir.dt.float32
BF16 = mybir.dt.bfloat16
F8E4 = mybir.dt.float8e4

AF = mybir.ActivationFunctionType
OP = mybir.AluOpType
RG = [list(range(N_CORES))]


def build_kernel():
    nc = bacc.Bacc("TRN2", target_bir_lowering=False, debug=False,
                   num_devices=N_CORES)

    xt = nc.dram_tensor("xt", [D_IN, B_SH], BF16, kind="ExternalInput")
    w8 = nc.dram_tensor("w8", [OG * 128, KT * 256], F8E4,
                        kind="ExternalInput")
    gamma = nc.dram_tensor("gamma", [1, D_OUT], F32, kind="ExternalInput")
    beta = nc.dram_tensor("beta", [1, D_OUT], F32, kind="ExternalInput")
    out = nc.dram_tensor("out", [D_OUT, B_SH], BF16, kind="ExternalOutput")

    with tile.TileContext(nc) as tc:
        with tc.tile_pool(name="persist", bufs=1) as persist, \
             tc.tile_pool(name="y2scr", bufs=3) as y2pool, \
             tc.tile_pool(name="stage", bufs=2) as stage_pool, \
             tc.tile_pool(name="scr4", bufs=2) as scr4_pool, \
             tc.tile_pool(name="psum", bufs=2, space="PSUM") as psum_pool, \
             tc.tile_pool(name="dram", bufs=1, space="DRAM") as dram:

            # ---- persistent SBUF tiles ----
            x_sb = [persist.tile([128, B_SH], BF16, name=f"x{it}")
                    for it in range(KT)]
            w_sb = [persist.tile([128, KT * 256], F8E4, name=f"w{g}")
                    for g in range(OG)]
            y_all = persist.tile([128, OB * B_SH], BF16)
            gam8 = persist.tile([128, OB], F32)
            bet8 = persist.tile([128, OB], F32)
            sy_cols = persist.tile([128, OB * BB], F32)
            sy2_cols = persist.tile([128, OB * BB], F32)
            stats2 = [persist.tile([128, 2], F32, name=f"st{ob}")
                      for ob in range(OB)]
            gs = [persist.tile([128, 2], F32, name=f"gs{ob}")
                  for ob in range(OB)]
            gsr = [persist.tile([128, 2 * N_CORES], F32, name=f"gr{ob}")
                   for ob in range(OB)]
            acm = [persist.tile([128, 2], F32, name=f"ac{ob}")
                   for ob in range(OB)]
            cscr = [persist.tile([128, 6], F32, name=f"cs{ob}")
                    for ob in range(OB)]

            cbi = [dram.tile([1, 256 * len(grp)], F32, name=f"cbi{gi}",
                             tag=f"cbi{gi}")
                   for gi, grp in enumerate(GROUPS)]
            cbo = [dram.tile([N_CORES, 256 * len(grp)], F32,
                             name=f"cbo{gi}", tag=f"cbo{gi}")
                   for gi, grp in enumerate(GROUPS)]

            # ---- loads: w group 0 first, then x stripes on both rails ----
            nc.sync.dma_start(w_sb[0][:], w8[0:128, :])
            nc.scalar.dma_start(
                gam8[:], gamma[0:1, :].rearrange("a (j p) -> (a p) j", p=128))
            nc.scalar.dma_start(
                bet8[:], beta[0:1, :].rearrange("a (j p) -> (a p) j", p=128))
            for it in range(KT):
                eng = nc.sync if it % 2 == 0 else nc.scalar
                eng.dma_start(x_sb[it][:], xt[it * 128:(it + 1) * 128, :])
            for g in range(1, OG):
                eng = nc.scalar if g % 2 == 0 else nc.sync
                eng.dma_start(w_sb[g][:], w8[g * 128:(g + 1) * 128, :])

            def drain_tile(ob, bb, ps):
                """PSUM -> y_all (bf16) + partial sums; split DVE/ACT."""
                t = ob * BB + bb
                yslice = y_all[:, ob * B_SH + bb * 512:
                               ob * B_SH + bb * 512 + 512]
                nc.vector.tensor_scalar(
                    out=yslice, in0=ps[:], scalar1=1.0, scalar2=0.0,
                    op0=OP.mult, op1=OP.add,
                    accum_out=sy_cols[:, t:t + 1])
                scr = y2pool.tile([128, 512], BF16, name=f"y2s{ob}{bb}",
                                  tag="y2")
                nc.scalar.activation(scr[:], ps[:], AF.Square,
                                     accum_out=sy2_cols[:, t:t + 1])

            def collapse_ob(ob):
                """4 bblk partials -> stats2[ob] = [sum_y | sum_y2]."""
                s4a = scr4_pool.tile([128, BB], F32, name=f"s4a{ob}",
                                     tag="s4a")
                nc.vector.tensor_scalar(
                    out=s4a[:], in0=sy_cols[:, ob * BB:(ob + 1) * BB],
                    scalar1=1.0, scalar2=0.0, op0=OP.mult, op1=OP.add,
                    accum_out=stats2[ob][:, 0:1])
                s4b = scr4_pool.tile([128, BB], F32, name=f"s4b{ob}",
                                     tag="s4b")
                nc.vector.tensor_scalar(
                    out=s4b[:], in0=sy2_cols[:, ob * BB:(ob + 1) * BB],
                    scalar1=1.0, scalar2=0.0, op0=OP.mult, op1=OP.add,
                    accum_out=stats2[ob][:, 1:2])

            def group_ar(gi):
                """ship the group's stats to DRAM and fire its AllReduce."""
                for idx, ob in enumerate(GROUPS[gi]):
                    nc.sync.dma_start(
                        cbi[gi][0:1, idx * 256:(idx + 1) * 256]
                        .rearrange("a (p j) -> (a p) j", p=128),
                        stats2[ob][:])
                nc.gpsimd.collective_compute(
                    "AllGather", OP.bypass, replica_groups=RG,
                    ins=[cbi[gi].opt()], outs=[cbo[gi].opt()])

            def finish_ob(gi, idx, ob):
                """read back global stats, coefficients, normalize, store."""
                eng = nc.sync if ob % 2 == 0 else nc.scalar
                eng.dma_start(
                    gsr[ob][:].rearrange("p (j r) -> p j r", j=2),
                    cbo[gi][:, idx * 256:(idx + 1) * 256]
                    .rearrange("r (p j) -> p j r", p=128))
                rsc = scr4_pool.tile([128, N_CORES], F32, name=f"rs{ob}",
                                     tag="rsc")
                nc.vector.tensor_scalar(
                    out=rsc[:], in0=gsr[ob][:, 0:N_CORES],
                    scalar1=1.0, scalar2=0.0, op0=OP.mult, op1=OP.add,
                    accum_out=gs[ob][:, 0:1])
                rsc2 = scr4_pool.tile([128, N_CORES], F32, name=f"rt{ob}",
                                      tag="rsc2")
                nc.vector.tensor_scalar(
                    out=rsc2[:], in0=gsr[ob][:, N_CORES:2 * N_CORES],
                    scalar1=1.0, scalar2=0.0, op0=OP.mult, op1=OP.add,
                    accum_out=gs[ob][:, 1:2])
                cs = cscr[ob]
                # mean, E[y^2]
                nc.vector.tensor_scalar_mul(cs[:, 0:1], gs[ob][:, 0:1],
                                            1.0 / B_TOT)
                nc.vector.tensor_scalar_mul(cs[:, 1:2], gs[ob][:, 1:2],
                                            1.0 / B_TOT)
                # var = E[y^2] - mean^2 + eps
                nc.vector.tensor_tensor(out=cs[:, 2:3], in0=cs[:, 0:1],
                                        in1=cs[:, 0:1], op=OP.mult)
                nc.vector.tensor_tensor(out=cs[:, 3:4], in0=cs[:, 1:2],
                                        in1=cs[:, 2:3], op=OP.subtract)
                nc.vector.tensor_scalar_add(cs[:, 3:4], cs[:, 3:4], BN_EPS)
                # inv = 1/sqrt(var)
                nc.scalar.activation(cs[:, 4:5], cs[:, 3:4], AF.Sqrt)
                nc.vector.reciprocal(cs[:, 4:5], cs[:, 4:5])
                # a = gamma * inv ; c = beta - mean * a
                nc.vector.tensor_tensor(out=acm[ob][:, 0:1],
                                        in0=gam8[:, ob:ob + 1],
                                        in1=cs[:, 4:5], op=OP.mult)
                nc.vector.tensor_tensor(out=cs[:, 5:6], in0=cs[:, 0:1],
                                        in1=acm[ob][:, 0:1], op=OP.mult)
                nc.vector.tensor_tensor(out=acm[ob][:, 1:2],
                                        in0=bet8[:, ob:ob + 1],
                                        in1=cs[:, 5:6], op=OP.subtract)
                stg = stage_pool.tile([128, B_SH], BF16, name=f"stg{ob}",
                                      tag="stg")
                nc.vector.tensor_scalar(
                    out=stg[:], in0=y_all[:, ob * B_SH:(ob + 1) * B_SH],
                    scalar1=acm[ob][:, 0:1], scalar2=acm[ob][:, 1:2],
                    op0=OP.mult, op1=OP.add)
                eng.dma_start(out[ob * 128:(ob + 1) * 128, :], stg[:])

            # ---- Phase A: obs 0,1 interleaved, stripe-outer so the PE
            # ---- consumes x at DMA arrival rate (8 banks live) ----
            psA = {}
            for ob in (0, 1):
                for bb in range(BB):
                    psA[(ob, bb)] = psum_pool.tile(
                        [128, 512], F32, name=f"psA{ob}{bb}", tag=f"a{bb}")
            for it in range(KT):
                for ob in (0, 1):
                    base = it * 256 + ob * 128
                    for bb in range(BB):
                        nc.tensor.matmul(
                            psA[(ob, bb)][:],
                            w_sb[0][:, base:base + 128],
                            x_sb[it][:, bb * 512:(bb + 1) * 512],
                            start=(it == 0), stop=(it == KT - 1))
            for ob in (0, 1):
                for bb in range(BB):
                    drain_tile(ob, bb, psA[(ob, bb)])
                collapse_ob(ob)

            # ---- Phase B: obs 2..7, bblk-outer (staggered drains) ----
            for ob in range(2, OB):
                g, half = divmod(ob, 2)
                for bb in range(BB):
                    ps = psum_pool.tile([128, 512], F32, name=f"ps{ob}{bb}",
                                        tag=f"a{bb}")
                    base = half * 128
                    for it in range(KT):
                        nc.tensor.matmul(
                            ps[:],
                            w_sb[g][:, it * 256 + base:it * 256 + base + 128],
                            x_sb[it][:, bb * 512:(bb + 1) * 512],
                            start=(it == 0), stop=(it == KT - 1))
                    drain_tile(ob, bb, ps)
                collapse_ob(ob)
                if ob == 2:
                    group_ar(0)
                elif ob == 4:
                    group_ar(1)
                elif ob == 6:
                    group_ar(2)
                elif ob == 7:
                    group_ar(3)
            # All finish work strictly after the last drain: coefficient ops
            # must never sit in an engine FIFO ahead of pending PSUM drains
            # (the ACT/DVE queues run far ahead of the tensor engine).
            for gi in range(len(GROUPS)):
                for idx, o in enumerate(GROUPS[gi]):
                    finish_ob(gi, idx, o)

    nc.compile()
    return nc


_NC_CACHE = None


def kernel(x, weight, bias, gamma, beta):
    global _NC_CACHE
    if _NC_CACHE is None:
        _NC_CACHE = build_kernel()
    nc = _NC_CACHE

    x = np.asarray(x, dtype=np.float32)
    weight = np.asarray(weight, dtype=np.float32)
    gamma = np.asarray(gamma, dtype=np.float32).reshape(1, D_OUT)
    beta = np.asarray(beta, dtype=np.float32).reshape(1, D_OUT)

    # sign(w).T in fp8 (+-1 exact): w8[g*128 + p, it*256 + oo] =
    # sign(w).T[it*128 + p, g*256 + oo]  (contiguous per-partition rows)
    wsT = np.where(weight >= 0, np.float32(1.0), np.float32(-1.0)).T
    w8 = np.ascontiguousarray(
        wsT.reshape(KT, 128, OG, 256).transpose(2, 1, 0, 3)
    ).reshape(OG * 128, KT * 256).astype(ml_dtypes.float8_e4m3)

    in_maps = []
    for i in range(N_CORES):
        shard = x[i * B_SH:(i + 1) * B_SH]          # [B_SH, D_IN]
        xt_i = np.ascontiguousarray(shard.T).astype(ml_dtypes.bfloat16)
        in_maps.append({
            "xt": xt_i,
            "w8": w8,
            "gamma": gamma,
            "beta": beta,
        })

    res = bass_utils.run_bass_kernel_spmd(
        nc, in_maps, core_ids=list(range(N_CORES)),
        trace=bool(int(os.environ.get("KERNEL_TRACE", "0"))),
    )
    kernel.last_results = res

    full = np.empty((B_TOT, D_OUT), dtype=np.float32)
    for i in range(N_CORES):
        y_ob = np.asarray(res.results[i]["out"])    # [D_OUT, B_SH] bf16
        full[i * B_SH:(i + 1) * B_SH] = y_ob.T.astype(np.float32)
    return full
